# revision 47
# baseline (speedup 1.0000x reference)
"""Trainium2 Bass kernel for a 3-block GPT (B=2,T=2048,E=1024,H=16,V=32000).

Sharding: block-cyclic sequence-parallel over 8 cores (2 groups of 4, one per
batch). Core j of a group owns query blocks {j, 4+j, 8+j, 12+j} (128 tokens
each). Causality then gives a program-static schedule: attention slot i needs
key blocks 0..4i+3 on every core; only the diagonal quad's mask is per-core
data. Matmuls run in fp8e4m3 DoubleRow mode (two 128-row contraction planes
per instruction, 0.5 cycles/row); weights are pre-scaled by 32 to clear the
fp8 subnormal range and descaled in the fused psum-readout ops. K/V are
gathered per-batch-group with a single fp8 AllGather per layer. lm_head runs
in bf16 for accuracy. Biases bo/b2 are folded into the residual operand,
b1 rides the relu fusion pre-scaled, blm is added on host.
"""

import numpy as np
import ml_dtypes
from contextlib import ExitStack

import concourse.bass as bass
import concourse.mybir as mybir
import concourse.tile as tile
from concourse import bacc
from concourse.masks import make_identity
from concourse import bass_utils

B, T, E, H, V = 2, 2048, 1024, 16, 32000
HD, L = 64, 3
NC = 8
S = 512                  # tokens per core
CH = 4                   # cores per group
FF = 4 * E
EPS = 1e-5
P = 128
NE = E // P              # 8 e-tiles
NHP = H // 2             # 8 head pairs
NVT = V // P             # 250 vocab tiles
NFT = FF // P            # 32 f-tiles
WS = 32.0                # fp8 weight pre-scale
F32 = mybir.dt.float32
BF16 = mybir.dt.bfloat16
FP8 = mybir.dt.float8e4
I32 = mybir.dt.int32
AF = mybir.ActivationFunctionType
OP = mybir.AluOpType
DRM = mybir.MatmulPerfMode.DoubleRow
E4NP = ml_dtypes.float8_e4m3fn

KBLOB = E * S            # fp8 elems of k per core chunk
VBLOB = S * E
CHUNK = KBLOB + VBLOB    # 1 MiB per chunk


def _ln(nc, pools, src, out_cb, g_t, b_t, ones_b, eps_t):
    """LayerNorm over E. src: 8 bf16 [128,S] tiles. out via out_cb(e, t2, g, b)
    which must emit the final affine (Act) writing wherever needed."""
    pacc, tp = pools["pacc"], pools["tp"]
    psm = pacc.tile([P, S], F32, tag="ps_a", name="ln_psm", bufs=2)
    pss = pacc.tile([P, S], F32, tag="ps_a", name="ln_pss", bufs=2)
    sqs = []
    for e in range(NE):
        sq = tp.tile([P, S], BF16, tag="ln_sq", name="ln_sq", bufs=3)
        nc.vector.tensor_tensor(out=sq[:], in0=src[e][:], in1=src[e][:], op=OP.mult)
        sqs.append(sq)
    for e in range(NE):
        nc.tensor.matmul(psm[:], lhsT=ones_b[:], rhs=src[e][:],
                         start=(e == 0), stop=(e == NE - 1), skip_group_check=True)
    for e in range(NE):
        nc.tensor.matmul(pss[:], lhsT=ones_b[:], rhs=sqs[e][:],
                         start=(e == 0), stop=(e == NE - 1), skip_group_check=True)
    mean = tp.tile([P, S], F32, tag="ln_mean", name="ln_mean")
    nc.vector.tensor_scalar(mean[:], psm[:], 1.0 / E, None, OP.mult)
    msq = tp.tile([P, S], F32, tag="ln_msq", name="ln_msq")
    nc.vector.tensor_tensor(out=msq[:], in0=mean[:], in1=mean[:], op=OP.mult)
    var = tp.tile([P, S], F32, tag="ln_var", name="ln_var")
    nc.vector.scalar_tensor_tensor(out=var[:], in0=pss[:], scalar=1.0 / E,
                                   in1=msq[:], op0=OP.mult, op1=OP.subtract)
    std = tp.tile([P, S], F32, tag="ln_std", name="ln_std")
    nc.scalar.activation(std[:], var[:], AF.Sqrt, bias=eps_t[:])
    rstd = tp.tile([P, S], F32, tag="ln_rstd", name="ln_rstd")
    nc.vector.reciprocal(rstd[:], std[:])
    meanb = tp.tile([P, S], BF16, tag="ln_meanb", name="ln_meanb")
    nc.vector.tensor_copy(meanb[:], mean[:])
    rstdb = tp.tile([P, S], BF16, tag="ln_rstdb", name="ln_rstdb")
    nc.vector.tensor_copy(rstdb[:], rstd[:])
    for e in range(NE):
        t = tp.tile([P, S], BF16, tag="ln_t", name="ln_t", bufs=3)
        nc.vector.tensor_tensor(out=t[:], in0=src[e][:], in1=meanb[:], op=OP.subtract)
        nc.vector.tensor_tensor(out=t[:], in0=t[:], in1=rstdb[:], op=OP.mult)
        out_cb(e, t, g_t[:, e:e + 1], b_t[:, e:e + 1])


def build_program():
    nc = bacc.Bacc("TRN2", target_bir_lowering=False, debug=False, num_devices=NC)

    # ---- DRAM I/O ----
    # fp8 DoubleRow weights: [P, m, kp, 2, P] (pre-scaled by 32)
    d_wq = nc.dram_tensor("wq8", [L, P, NE, NE // 2, 2, P], FP8, kind="ExternalInput")
    d_wk = nc.dram_tensor("wk8", [L, P, NE, NE // 2, 2, P], FP8, kind="ExternalInput")
    d_wo = nc.dram_tensor("wob", [L, P, NE, NE, P], BF16, kind="ExternalInput")
    d_wv = nc.dram_tensor("wvb", [L, P, NE, E], BF16, kind="ExternalInput")
    d_w1 = nc.dram_tensor("w1b", [L, P, NFT, NE, P], BF16, kind="ExternalInput")
    d_w2 = nc.dram_tensor("w2b", [L, P, NE, NFT, P], BF16, kind="ExternalInput")
    d_wlm = nc.dram_tensor("wlm2", [NVT // 2, P, 2, NE, P], BF16,
                           kind="ExternalInput")
    d_emb = nc.dram_tensor("emb", [V, E], F32, kind="ExternalInput")
    d_idx = nc.dram_tensor("idx", [S], I32, kind="ExternalInput")
    d_msk = nc.dram_tensor("maskd", [4, P, 4, P], FP8, kind="ExternalInput")
    d_ln1g = nc.dram_tensor("ln1g", [L, E], F32, kind="ExternalInput")
    d_ln1b = nc.dram_tensor("ln1b", [L, E], F32, kind="ExternalInput")
    d_ln2g = nc.dram_tensor("ln2g", [L, E], F32, kind="ExternalInput")
    d_ln2b = nc.dram_tensor("ln2b", [L, E], F32, kind="ExternalInput")
    d_bo = nc.dram_tensor("bo", [L, E], F32, kind="ExternalInput")
    d_b1 = nc.dram_tensor("b1", [L, FF], F32, kind="ExternalInput")
    d_b2 = nc.dram_tensor("b2", [L, E], F32, kind="ExternalInput")
    d_lnfg = nc.dram_tensor("lnfg", [E], F32, kind="ExternalInput")
    d_lnfb = nc.dram_tensor("lnfb", [E], F32, kind="ExternalInput")
    d_out = nc.dram_tensor("logt", [NVT, P, S], BF16, kind="ExternalOutput")

    groups = [[0, 1, 2, 3], [4, 5, 6, 7]]

    with ExitStack() as ctx:
        tc = ctx.enter_context(tile.TileContext(nc, num_cores=NC))
        const = ctx.enter_context(tc.tile_pool(name="const", bufs=1))
        pp_x = ctx.enter_context(tc.tile_pool(name="xres", bufs=1))
        pp_pl = ctx.enter_context(tc.tile_pool(name="planes", bufs=1))
        tp = ctx.enter_context(tc.tile_pool(name="tp", bufs=1))
        wp = ctx.enter_context(tc.tile_pool(name="wstream", bufs=1))
        ap_p = ctx.enter_context(tc.tile_pool(name="attn", bufs=1))
        pacc = ctx.enter_context(tc.tile_pool(name="pacc", bufs=1, space="PSUM"))
        dram = ctx.enter_context(tc.tile_pool(name="ccdram", bufs=2, space="DRAM"))
        pools = {"pacc": pacc, "tp": tp}

        ident = const.tile([P, P], F32, name="ident")
        make_identity(nc, ident[:])
        ones_b = const.tile([P, P], BF16, name="ones_b")
        nc.vector.memset(ones_b[:], 1.0)
        ones8 = const.tile([P, 2, HD], FP8, name="ones8")
        nc.vector.memset(ones8[:], 1.0)
        zeros_b = const.tile([P, S], BF16, name="zeros_b")
        nc.vector.memset(zeros_b[:], 0.0)
        eps_t = const.tile([P, 1], F32, name="eps_t")
        nc.vector.memset(eps_t[:], EPS)

        def ldvec(dt_ap, n, name):
            t = const.tile([P, n], F32, tag=name, name=name)
            nc.sync.dma_start(out=t[:], in_=dt_ap.rearrange("(a p) -> p a", p=P))
            return t

        t_ln1g = [ldvec(d_ln1g.ap()[l], NE, f"ln1g{l}") for l in range(L)]
        t_ln1b = [ldvec(d_ln1b.ap()[l], NE, f"ln1b{l}") for l in range(L)]
        t_ln2g = [ldvec(d_ln2g.ap()[l], NE, f"ln2g{l}") for l in range(L)]
        t_ln2b = [ldvec(d_ln2b.ap()[l], NE, f"ln2b{l}") for l in range(L)]
        t_bo = [ldvec(d_bo.ap()[l], NE, f"bo{l}") for l in range(L)]
        t_b1 = [ldvec(d_b1.ap()[l], NFT, f"b1{l}") for l in range(L)]
        t_b2 = [ldvec(d_b2.ap()[l], NE, f"b2{l}") for l in range(L)]
        t_lnfg = ldvec(d_lnfg.ap(), NE, "lnfg")
        t_lnfb = ldvec(d_lnfb.ap(), NE, "lnfb")

        # diagonal-quad masks (per-core data): [slot, 128k, 4kb, 128q]
        mask_t = []
        for i in range(4):
            m = const.tile([P, 8, P], FP8, tag=f"mskd{i}", name=f"mskd{i}")
            nc.sync.dma_start(out=m[:, 0:4], in_=d_msk.ap()[i])
            nc.sync.dma_start(out=m[:, 4:8], in_=d_msk.ap()[i])
            mask_t.append(m)

        # residual stream: 8 bf16 [128, S] tiles
        xT = [pp_x.tile([P, S], BF16, tag=f"x{e}", name=f"x{e}") for e in range(NE)]

        # ---- embedding gather + transpose ----
        idx_t = const.tile([P, S // P], I32, name="idx_t")
        nc.sync.dma_start(out=idx_t[:], in_=d_idx.ap().rearrange("(g p) -> p g", p=P))
        for g in range(S // P):
            xg = tp.tile([P, E], F32, tag="embg", name="embg", bufs=1)
            nc.gpsimd.indirect_dma_start(
                out=xg[:], out_offset=None, in_=d_emb.ap(),
                in_offset=bass.IndirectOffsetOnAxis(ap=idx_t[:, g:g + 1], axis=0))
            for e in range(NE):
                pst = pacc.tile([P, S], F32, tag="ps_a", name="tpose", bufs=2)
                nc.tensor.transpose(pst[:, 0:P], xg[:, e * P:(e + 1) * P], ident[:])
                nc.vector.tensor_copy(xT[e][:, g * P:(g + 1) * P], pst[:, 0:P])

        # x fp8 planes for matmul rhs: 4 tiles [128, 2, S]
        # x planes carry 8*x so layer-0 embeddings (~0.02) clear the fp8
        # subnormal range; the 1/8 rides the existing k/q/v descale copies
        xP = [pp_pl.tile([P, 2, S], FP8, tag=f"xp{r}", name=f"xp{r}")
              for r in range(NE // 2)]
        for e in range(NE):
            nc.vector.tensor_scalar(xP[e // 2][:, e % 2, :], xT[e][:], 8.0,
                                    None, OP.mult)

        def proj_dr(d_w, l, rhs_planes, nm):
            """W-stationary fp8 DR projection -> list of psum [128,S] per m."""
            w = wp.tile([P, NE, NE // 2, 2, P], FP8, tag="wproj", name=f"w{nm}",
                        bufs=2)
            nc.sync.dma_start(out=w[:], in_=d_w.ap()[l])
            outs = []
            for m in range(NE):
                ps = pacc.tile([P, S], F32, tag="ps_a", name=f"ps{nm}", bufs=2)
                for kp in range(NE // 2):
                    nc.tensor.matmul(ps[:], lhsT=w[:, m, kp], rhs=rhs_planes[kp][:],
                                     start=(kp == 0), stop=(kp == NE // 2 - 1),
                                     perf_mode=DRM)
                outs.append(ps)
            return outs

        # ---- transformer layers ----
        for l in range(L):
            with tc.tile_pool(name="kv", bufs=1) as kvp, \
                 tc.tile_pool(name="qq", bufs=1) as qp, \
                 tc.tile_pool(name="oo", bufs=1) as op_:
                # K projection -> fp8 (x 1/32) -> per-half kv blobs
                # (half A = local token cols 0:256 = abs key quads 0,1)
                KB2 = E * (S // 2)
                CH2 = 2 * KB2
                kvin = [dram.tile([CH2], FP8, tag=f"kvin{a}", name=f"kvin{a}")
                        for a in range(2)]
                kreg = [kvin[a][:KB2].rearrange("(r s) -> r s", r=E, s=S // 2)
                        for a in range(2)]
                vreg = [kvin[a][KB2:].rearrange("(t e) -> t e", t=S // 2, e=E)
                        for a in range(2)]
                # k blob rows in consumer order (row = 8p + 2hh + i, with the
                # Wk output columns host-permuted to match) so each m-tile
                # writes one contiguous row block per half
                kps = proj_dr(d_wk, l, xP, "k")
                for m in range(NE):
                    kl = tp.tile([P, S], FP8, tag="kl", name="kl", bufs=2)
                    nc.vector.tensor_scalar(kl[:], kps[m][:], 1.0 / (WS * 8), None, OP.mult)
                    for a in range(2):
                        nc.sync.dma_start(
                            out=kreg[a][m * P:(m + 1) * P, :],
                            in_=kl[:, a * (S // 2):(a + 1) * (S // 2)])
                # V projection half A (token blocks 0,1) -> AllGather A
                wv = wp.tile([P, NE, E], BF16, tag="wv", name="wv", bufs=1)
                nc.sync.dma_start(out=wv[:], in_=d_wv.ap()[l])
                kvout = [dram.tile([CH, CH2], FP8, tag=f"kvout{a}",
                                   name=f"kvout{a}") for a in range(2)]

                def vproj_half(a):
                    for tb in (2 * a, 2 * a + 1):
                        for hf in range(2):
                            ps = pacc.tile([P, S], F32, tag="ps_a", name="psv",
                                           bufs=2)
                            for k in range(NE):
                                nc.tensor.matmul(
                                    ps[:], lhsT=xT[k][:, tb * P:(tb + 1) * P],
                                    rhs=wv[:, k, hf * S:(hf + 1) * S],
                                    start=(k == 0), stop=(k == NE - 1))
                            vl = tp.tile([P, S], FP8, tag="vl", name="vl", bufs=2)
                            nc.vector.tensor_scalar(vl[:], ps[:], 8.0,
                                                    None, OP.mult)
                            nc.sync.dma_start(
                                out=vreg[a][(tb % 2) * P:(tb % 2 + 1) * P,
                                            hf * S:(hf + 1) * S],
                                in_=vl[:])

                def gather_half(a):
                    nc.gpsimd.collective_compute(
                        "AllGather", OP.bypass, replica_groups=groups,
                        ins=[kvin[a][:]], outs=[kvout[a][:]])

                ktile = ap_p.tile([P, 4, 2, 4, 4, P], FP8, tag="kt", name="kt",
                                  bufs=1)
                vt = [ap_p.tile([P, 2, E], FP8, tag=f"vt{r}", name=f"vt{r}",
                                bufs=1) for r in range(NE)]

                def load_half(a):
                    # k/v slab loads on the Act DMA queue so they don't queue
                    # behind SP-issued q/blob writes
                    kr = kvout[a][:, :KB2].rearrange(
                        "c (p hh i us) -> c p hh i us", p=P, hh=4, i=2,
                        us=S // 2)
                    for c in range(4):
                        nc.gpsimd.dma_start(
                            out=ktile[:, :, :, c, 2 * a:2 * a + 2],
                            in_=kr[c].rearrange("p hh i (u s) -> p hh i u s",
                                                u=2, s=P))
                    for r in range(4 * a, 4 * a + 4):
                        for pl in range(2):
                            kb = 2 * r + pl
                            u, c = kb // 4, kb % 4
                            u2 = u % 2
                            nc.gpsimd.dma_start(
                                out=vt[r][:, pl],
                                in_=kvout[a][c,
                                             KB2 + u2 * P * E:KB2 + (u2 + 1) * P * E]
                                .rearrange("(p s) -> p s", p=P))

                vproj_half(0)
                gather_half(0)

                # Q projection (PE busy during AllGather A) -> DRAM roundtrip
                # for the [32, 2, q] plane layout
                qbuf = dram.tile([E, S], FP8, tag="qbuf", name="qbuf")
                qps = proj_dr(d_wq, l, xP, "q")
                for m in range(NE):
                    ql = tp.tile([P, S], FP8, tag="ql", name="ql", bufs=2)
                    nc.vector.tensor_scalar(ql[:], qps[m][:], 1.0 / (WS * 8),
                                            None, OP.mult)
                    nc.sync.dma_start(out=qbuf[m * P:(m + 1) * P, :], in_=ql[:])
                # q tile [128 (4g x 32d), 4hh, 2i, 512]; head h = 4*hh + g;
                # blob row = 8p + 2hh + i -> single balanced DMA
                qtile = qp.tile([P, 4, 2, S], FP8, tag="qt", name="qt")
                nc.sync.dma_start(
                    out=qtile[:],
                    in_=qbuf[:].rearrange("(p hh i) s -> p hh i s",
                                          p=P, hh=4, i=2))

                vproj_half(1)
                load_half(0)
                gather_half(1)
                load_half(1)

                # attention: slot-outer so quad-0/1 work overlaps AllGather B.
                # Head pairs (h, h+8) share a partition group -> one batched
                # exp per quad pair (halves the per-instruction PSUM-access
                # penalty on the Act engine).
                oT = [op_.tile([P, 4, P], BF16, tag=f"ot{m}", name=f"ot{m}")
                      for m in range(NE)]
                og_t = [op_.tile([64, 4, P], BF16, tag=f"og{oh}", name=f"og{oh}")
                        for oh in range(NHP)]
                for i in range(4):
                    nkbp = 2 * (i + 1)
                    for hp2 in range(NHP):
                        g, hhl = hp2 % 4, hp2 // 4
                        pd = [pacc.tile([64, 2, P], F32, tag="ps_v",
                                        name="pavden", bufs=2) for _ in range(2)]
                        for u in range(i + 1):
                            sc = pacc.tile([P, 8, P], F32, tag="ps_s",
                                           name="sc", bufs=2)
                            for half in range(2):
                                hh = hhl + 2 * half
                                for c in range(4):
                                    nc.tensor.matmul(
                                        sc[:, 4 * half + c],
                                        lhsT=ktile[32 * g:32 * (g + 1), hh, :, c, u],
                                        rhs=qtile[32 * g:32 * (g + 1), hh, :,
                                                  i * P:(i + 1) * P],
                                        start=True, stop=True, perf_mode=DRM,
                                        tile_position=(32 * g, 0))
                            pa = ap_p.tile([P, 8, P], FP8, tag="pa",
                                           name="pa", bufs=4)
                            nc.scalar.activation(pa[:], sc[:], AF.Exp,
                                                 scale=HD ** -0.5)
                            if u == i:
                                nc.gpsimd.tensor_tensor(
                                    out=pa[:], in0=pa[:], in1=mask_t[i][:],
                                    op=OP.mult)
                            for half in range(2):
                                h = hp2 + 8 * half
                                for r2 in range(2):
                                    kbp = 2 * u + r2
                                    first = (kbp == 0)
                                    last = (kbp == nkbp - 1)
                                    nc.tensor.matmul(
                                        pd[half][:, 0],
                                        lhsT=vt[kbp][:, :, h * HD:(h + 1) * HD],
                                        rhs=pa[:, 4 * half + 2 * r2:
                                               4 * half + 2 * r2 + 2],
                                        start=first, stop=last, perf_mode=DRM,
                                        skip_group_check=True)
                                    nc.tensor.matmul(
                                        pd[half][:, 1],
                                        lhsT=ones8[:, :, :],
                                        rhs=pa[:, 4 * half + 2 * r2:
                                               4 * half + 2 * r2 + 2],
                                        start=first, stop=last, perf_mode=DRM,
                                        skip_group_check=True)
                        for half in range(2):
                            h = hp2 + 8 * half
                            rec = tp.tile([64, P], F32, tag="rec", name="rec",
                                          bufs=2)
                            nc.vector.reciprocal(rec[:], pd[half][:, 1])
                            # o = pav*rec -> bf16; odd heads stage + DMA-shift
                            # (DR matmuls must write psum partition base 0)
                            if h % 2 == 0:
                                nc.vector.tensor_tensor(
                                    out=oT[h // 2][0:64, i],
                                    in0=pd[half][:, 0], in1=rec[:], op=OP.mult)
                            else:
                                nc.vector.tensor_tensor(
                                    out=og_t[h // 2][:, i],
                                    in0=pd[half][:, 0], in1=rec[:], op=OP.mult)
                for h in range(1, H, 2):
                    nc.sync.dma_start(out=oT[h // 2][64:128], in_=og_t[h // 2][:])

                # output projection (bf16 for accuracy) + bias + residual
                sum_t = []
                wo = None
                for m in range(NE):
                    if m % 4 == 0:
                        wo = wp.tile([P, 4, NE, P], BF16, tag="w1",
                                     name="wo", bufs=2)
                        nc.sync.dma_start(out=wo[:], in_=d_wo.ap()[l, :, m:m + 4])
                    ps = pacc.tile([P, S], F32, tag="ps_a", name="pso", bufs=2)
                    for k in range(NE):
                        nc.tensor.matmul(ps[:], lhsT=wo[:, m % 4, k],
                                         rhs=oT[k][:], start=(k == 0),
                                         stop=(k == NE - 1))
                    xb = tp.tile([P, S], BF16, tag="xb", name="xb", bufs=3)
                    nc.vector.tensor_scalar(xb[:], xT[m][:], t_bo[l][:, m:m + 1],
                                            None, OP.add)
                    st = op_.tile([P, S], BF16, tag=f"st{m}", name=f"st{m}")
                    nc.vector.scalar_tensor_tensor(
                        out=st[:], in0=ps[:], scalar=0.125, in1=xb[:],
                        op0=OP.mult, op1=OP.add)
                    sum_t.append(st)

                def ln1_out(e, t2, g, b):
                    nc.scalar.activation(xT[e][:], t2[:], AF.Identity, bias=b, scale=g)
                _ln(nc, pools, sum_t, ln1_out, t_ln1g[l], t_ln1b[l], ones_b, eps_t)

            # FFN (bf16 for accuracy: fp8 here injects ~5%/layer into the
            # residual, which blows the 2e-2 budget)
            with tc.tile_pool(name="ht", bufs=1) as hp_:
                hT = [hp_.tile([P, S], BF16, tag=f"h{f2}", name=f"h{f2}")
                      for f2 in range(NFT)]
                w1 = None
                for f in range(NFT):
                    if f % 4 == 0:
                        w1 = wp.tile([P, 4, NE, P], BF16, tag="w1",
                                     name="w1", bufs=2)
                        nc.sync.dma_start(out=w1[:],
                                          in_=d_w1.ap()[l, :, f:f + 4])
                    ps = pacc.tile([P, S], F32, tag="ps_a", name="psf", bufs=2)
                    for k in range(NE):
                        nc.tensor.matmul(ps[:], lhsT=w1[:, f % 4, k],
                                         rhs=xT[k][:], start=(k == 0),
                                         stop=(k == NE - 1))
                    nc.vector.scalar_tensor_tensor(
                        out=hT[f][:], in0=ps[:],
                        scalar=t_b1[l][:, f:f + 1], in1=zeros_b[:],
                        op0=OP.add, op1=OP.max)
                w2 = None
                sum2 = []
                for m in range(NE):
                    if True:
                        w2 = wp.tile([P, 1, NFT, P], BF16, tag="w2",
                                     name="w2", bufs=2)
                        nc.sync.dma_start(out=w2[:],
                                          in_=d_w2.ap()[l, :, m:m + 1])
                    ps = pacc.tile([P, S], F32, tag="ps_a", name="ps2", bufs=2)
                    for k in range(NFT):
                        nc.tensor.matmul(ps[:], lhsT=w2[:, 0, k],
                                         rhs=hT[k][:], start=(k == 0),
                                         stop=(k == NFT - 1))
                    xb = tp.tile([P, S], BF16, tag="xb2", name="xb2", bufs=3)
                    nc.vector.tensor_scalar(xb[:], xT[m][:], t_b2[l][:, m:m + 1],
                                            None, OP.add)
                    st = tp.tile([P, S], BF16, tag=f"s2{m}", name=f"s2{m}")
                    nc.vector.scalar_tensor_tensor(
                        out=st[:], in0=ps[:], scalar=1.0, in1=xb[:],
                        op0=OP.mult, op1=OP.add)
                    sum2.append(st)

                if l < L - 1:
                    def ln2_out(e, t2, g, b):
                        nc.scalar.activation(xT[e][:], t2[:], AF.Identity, bias=b,
                                             scale=g)
                        nc.vector.tensor_scalar(xP[e // 2][:, e % 2, :],
                                                xT[e][:], 8.0, None, OP.mult)
                    _ln(nc, pools, sum2, ln2_out, t_ln2g[l], t_ln2b[l], ones_b,
                        eps_t)
                else:
                    # fuse LN2(last) and LNf: LN(LN(x)) with composed affine is
                    # NOT equal in general; do them separately.
                    def ln2_out(e, t2, g, b):
                        nc.scalar.activation(xT[e][:], t2[:], AF.Identity, bias=b,
                                             scale=g)
                    _ln(nc, pools, sum2, ln2_out, t_ln2g[l], t_ln2b[l], ones_b,
                        eps_t)

        # ---- final LN + lm_head (bf16) ----
        with tc.tile_pool(name="lmx", bufs=1) as lmp:
            xB = [lmp.tile([P, S], BF16, tag=f"xlm{e}", name=f"xlm{e}")
                  for e in range(NE)]

            def lnf_out(e, t2, g, b):
                nc.scalar.activation(xB[e][:], t2[:], AF.Identity, bias=b, scale=g)
            _ln(nc, pools, xT, lnf_out, t_lnfg, t_lnfb, ones_b, eps_t)

            for vg in range(NVT // 2):
                w = wp.tile([P, 2, NE, P], BF16, tag="wlm", name="wlm", bufs=2)
                nc.sync.dma_start(out=w[:], in_=d_wlm.ap()[vg])
                lg = tp.tile([P, 2, S], BF16, tag="lg", name="lg", bufs=2)
                for v2 in range(2):
                    ps = pacc.tile([P, S], F32, tag="ps_a", name="pslm", bufs=2)
                    for k in range(NE):
                        nc.tensor.matmul(ps[:], lhsT=w[:, v2, k], rhs=xB[k][:],
                                         start=(k == 0), stop=(k == NE - 1))
                    if v2 == 0:
                        nc.vector.tensor_copy(lg[:, v2], ps[:])
                    else:
                        nc.scalar.copy(lg[:, v2], ps[:])
                nc.sync.dma_start(out=d_out.ap()[2 * vg:2 * vg + 2]
                                  .rearrange("v p s -> p v s"), in_=lg[:])

    nc.compile()
    return nc


_CACHED = {}


def _pack_dr_w(w, m_tiles):
    """[E_in, E_out] -> [P, m, kp, 2, P] fp8 with x32 prescale."""
    ein, eout = w.shape
    kp = ein // 256
    arr = (w * WS).reshape(kp, 2, P, m_tiles, P)
    # w[kpair*256 + i*128 + p, m*128 + c] -> [p, m, kp, i, c]
    arr = arr.transpose(2, 3, 0, 1, 4)
    return np.ascontiguousarray(arr).astype(E4NP)


def _prep_weights(inputs):
    f32 = np.float32
    Wq, Wk, Wv = (np.asarray(inputs[k], dtype=f32) for k in ("Wq", "Wk", "Wv"))
    # [L,H,E,HD] -> [L, E, H*HD]
    wq = Wq.transpose(0, 2, 1, 3).reshape(L, E, E)
    wk = Wk.transpose(0, 2, 1, 3).reshape(L, E, E)
    wv = Wv.transpose(0, 2, 1, 3).reshape(L, E, E)
    Wo = np.asarray(inputs["Wo"], dtype=f32)
    W1 = np.asarray(inputs["W1"], dtype=f32)
    W2 = np.asarray(inputs["W2"], dtype=f32)
    Wlm = np.asarray(inputs["Wlm"], dtype=f32)

    # permute k/q output features into blob-row order r = 8p + 2hh + i
    r = np.arange(E)
    p_, q8 = r // 8, r % 8
    hh_, i_ = q8 // 2, q8 % 2
    g_, d_ = p_ // 32, p_ % 32
    perm = (4 * hh_ + g_) * 64 + i_ * 32 + d_
    wq8 = np.stack([_pack_dr_w(wq[l][:, perm], NE) for l in range(L)])
    wk8 = np.stack([_pack_dr_w(wk[l][:, perm], NE) for l in range(L)])
    bf = ml_dtypes.bfloat16
    # Wv as moving rhs: [p, k, col]
    wvb = np.stack([np.ascontiguousarray(
        wv[l].reshape(NE, P, E).transpose(1, 0, 2)).astype(bf)
        for l in range(L)])
    wob = np.stack([np.ascontiguousarray(
        Wo[l].reshape(NE, P, NE, P).transpose(1, 2, 0, 3)).astype(bf)
        for l in range(L)])
    # W1 [E, FF] -> [P, f, k, P]; W2 [FF, E] -> [P, m, k, P]
    w1b = np.stack([np.ascontiguousarray(
        W1[l].reshape(NE, P, NFT, P).transpose(1, 2, 0, 3)).astype(bf)
        for l in range(L)])
    w2b = np.stack([np.ascontiguousarray(
        W2[l].reshape(NFT, P, NE, P).transpose(1, 2, 0, 3)).astype(bf)
        for l in range(L)])
    # Wlm [E, V] -> [vg, P, 2, k, P]
    wlm2 = np.ascontiguousarray(
        Wlm.reshape(NE, P, NVT // 2, 2, P).transpose(2, 1, 3, 0, 4)).astype(bf)

    return {
        "wq8": wq8, "wk8": wk8, "wob": wob, "wvb": wvb, "w1b": w1b, "w2b": w2b,
        "wlm2": wlm2,
        "emb": np.ascontiguousarray(inputs["emb"]).astype(f32),
        "ln1g": np.ascontiguousarray(inputs["ln1_g"]).astype(f32),
        "ln1b": np.ascontiguousarray(inputs["ln1_b"]).astype(f32),
        "ln2g": np.ascontiguousarray(inputs["ln2_g"]).astype(f32),
        "ln2b": np.ascontiguousarray(inputs["ln2_b"]).astype(f32),
        "bo": np.ascontiguousarray(inputs["bo"]).astype(f32),
        "b1": np.ascontiguousarray(inputs["b1"]).astype(f32),
        "b2": np.ascontiguousarray(inputs["b2"]).astype(f32),
        "lnfg": np.ascontiguousarray(inputs["lnf_g"]).astype(f32),
        "lnfb": np.ascontiguousarray(inputs["lnf_b"]).astype(f32),
    }


def kernel(**inputs):
    if "nc" not in _CACHED:
        _CACHED["nc"] = build_program()
    nc = _CACHED["nc"]

    shared = _prep_weights(inputs)
    index = np.asarray(inputs["index"])

    kpos = np.arange(P)
    in_maps = []
    perms = []
    for c in range(NC):
        b, j = c // CH, c % CH
        perm = np.concatenate([np.arange((4 * i + j) * P, (4 * i + j + 1) * P)
                               for i in range(4)])
        perms.append(perm)
        # diag-quad masks [slot, k, kb_in_quad (c2), q]
        m = np.zeros((4, P, 4, P), np.float32)
        for i in range(4):
            for c2 in range(4):
                if c2 < j:
                    m[i, :, c2, :] = 1.0
                elif c2 == j:
                    m[i, :, c2, :] = (kpos[:, None] <= kpos[None, :])
        im = dict(shared)
        im["maskd"] = m.astype(E4NP)
        im["idx"] = np.ascontiguousarray(index[b, perm]).astype(np.int32)
        in_maps.append(im)

    res = bass_utils.run_bass_kernel_spmd(nc, in_maps, core_ids=list(range(NC)))
    blm = np.asarray(inputs["blm"], dtype=np.float32)
    out = np.zeros((B, T, V), np.float32)
    for c in range(NC):
        b = c // CH
        lg = res.results[c]["logt"].reshape(V, S).astype(np.float32)
        out[b, perms[c], :] = lg.T
    out += blm[None, None, :]
    return out


# revision 48
# speedup vs baseline: 1.1196x; 1.1196x over previous
"""Trainium2 Bass kernel for a 3-block GPT (B=2,T=2048,E=1024,H=16,V=32000).

Sharding: block-cyclic sequence-parallel over 8 cores (2 groups of 4, one per
batch). Core j of a group owns query blocks {j, 4+j, 8+j, 12+j} (128 tokens
each). Causality then gives a program-static schedule: attention slot i needs
key blocks 0..4i+3 on every core; only the diagonal quad's mask is per-core
data. Matmuls run in fp8e4m3 DoubleRow mode (two 128-row contraction planes
per instruction, 0.5 cycles/row); weights are pre-scaled by 32 to clear the
fp8 subnormal range and descaled in the fused psum-readout ops. K/V are
gathered per-batch-group with a single fp8 AllGather per layer. lm_head runs
in bf16 for accuracy. Biases bo/b2 are folded into the residual operand,
b1 rides the relu fusion pre-scaled, blm is added on host.
"""

import numpy as np
import ml_dtypes
from contextlib import ExitStack

import concourse.bass as bass
import concourse.mybir as mybir
import concourse.tile as tile
from concourse import bacc
from concourse.masks import make_identity
from concourse import bass_utils

B, T, E, H, V = 2, 2048, 1024, 16, 32000
HD, L = 64, 3
NC = 8
S = 512                  # tokens per core
CH = 4                   # cores per group
FF = 4 * E
EPS = 1e-5
P = 128
NE = E // P              # 8 e-tiles
NHP = H // 2             # 8 head pairs
NVT = V // P             # 250 vocab tiles
NFT = FF // P            # 32 f-tiles
WS = 32.0                # fp8 weight pre-scale
F32 = mybir.dt.float32
BF16 = mybir.dt.bfloat16
FP8 = mybir.dt.float8e4
I32 = mybir.dt.int32
AF = mybir.ActivationFunctionType
OP = mybir.AluOpType
DRM = mybir.MatmulPerfMode.DoubleRow
E4NP = ml_dtypes.float8_e4m3fn

KBLOB = E * S            # fp8 elems of k per core chunk
VBLOB = S * E
CHUNK = KBLOB + VBLOB    # 1 MiB per chunk


def _ln(nc, pools, src, out_cb, g_t, b_t, ones_b, eps_t):
    """LayerNorm over E. src: 8 bf16 [128,S] tiles. out via out_cb(e, t2, g, b)
    which must emit the final affine (Act) writing wherever needed."""
    pacc, tp = pools["pacc"], pools["tp"]
    psm = pacc.tile([P, S], F32, tag="ps_a", name="ln_psm", bufs=2)
    pss = pacc.tile([P, S], F32, tag="ps_a", name="ln_pss", bufs=2)
    sqs = []
    for e in range(NE):
        sq = tp.tile([P, S], BF16, tag="ln_sq", name="ln_sq", bufs=3)
        nc.vector.tensor_tensor(out=sq[:], in0=src[e][:], in1=src[e][:], op=OP.mult)
        sqs.append(sq)
    for e in range(NE):
        nc.tensor.matmul(psm[:], lhsT=ones_b[:], rhs=src[e][:],
                         start=(e == 0), stop=(e == NE - 1), skip_group_check=True)
    for e in range(NE):
        nc.tensor.matmul(pss[:], lhsT=ones_b[:], rhs=sqs[e][:],
                         start=(e == 0), stop=(e == NE - 1), skip_group_check=True)
    mean = tp.tile([P, S], F32, tag="ln_mean", name="ln_mean")
    nc.vector.tensor_scalar(mean[:], psm[:], 1.0 / E, None, OP.mult)
    msq = tp.tile([P, S], F32, tag="ln_msq", name="ln_msq")
    nc.vector.tensor_tensor(out=msq[:], in0=mean[:], in1=mean[:], op=OP.mult)
    var = tp.tile([P, S], F32, tag="ln_var", name="ln_var")
    nc.vector.scalar_tensor_tensor(out=var[:], in0=pss[:], scalar=1.0 / E,
                                   in1=msq[:], op0=OP.mult, op1=OP.subtract)
    std = tp.tile([P, S], F32, tag="ln_std", name="ln_std")
    nc.scalar.activation(std[:], var[:], AF.Sqrt, bias=eps_t[:])
    rstd = tp.tile([P, S], F32, tag="ln_rstd", name="ln_rstd")
    nc.vector.reciprocal(rstd[:], std[:])
    meanb = tp.tile([P, S], BF16, tag="ln_meanb", name="ln_meanb")
    nc.vector.tensor_copy(meanb[:], mean[:])
    rstdb = tp.tile([P, S], BF16, tag="ln_rstdb", name="ln_rstdb")
    nc.vector.tensor_copy(rstdb[:], rstd[:])
    for e in range(NE):
        t = tp.tile([P, S], BF16, tag="ln_t", name="ln_t", bufs=3)
        nc.vector.tensor_tensor(out=t[:], in0=src[e][:], in1=meanb[:], op=OP.subtract)
        nc.vector.tensor_tensor(out=t[:], in0=t[:], in1=rstdb[:], op=OP.mult)
        out_cb(e, t, g_t[:, e:e + 1], b_t[:, e:e + 1])


def build_program():
    nc = bacc.Bacc("TRN2", target_bir_lowering=False, debug=False, num_devices=NC)

    # ---- DRAM I/O ----
    # fp8 DoubleRow weights: [P, m, kp, 2, P] (pre-scaled by 32)
    d_wq = nc.dram_tensor("wq8", [L, P, NE, NE // 2, 2, P], FP8, kind="ExternalInput")
    d_wk = nc.dram_tensor("wk8", [L, P, NE, NE // 2, 2, P], FP8, kind="ExternalInput")
    d_wo = nc.dram_tensor("wob", [L, P, NE, NE, P], BF16, kind="ExternalInput")
    d_wv = nc.dram_tensor("wvb", [L, P, NE, E], BF16, kind="ExternalInput")
    d_w1 = nc.dram_tensor("w1b", [L, P, NFT, NE, P], BF16, kind="ExternalInput")
    d_w2 = nc.dram_tensor("w2b", [L, P, NE, NFT, P], BF16, kind="ExternalInput")
    d_wlm = nc.dram_tensor("wlm2", [NVT // 2, P, 2, NE, P], BF16,
                           kind="ExternalInput")
    d_emb = nc.dram_tensor("emb", [V, E], F32, kind="ExternalInput")
    d_idx = nc.dram_tensor("idx", [S], I32, kind="ExternalInput")
    d_msk = nc.dram_tensor("maskd", [4, P, 4, P], FP8, kind="ExternalInput")
    d_ln1g = nc.dram_tensor("ln1g", [L, E], F32, kind="ExternalInput")
    d_ln1b = nc.dram_tensor("ln1b", [L, E], F32, kind="ExternalInput")
    d_ln2g = nc.dram_tensor("ln2g", [L, E], F32, kind="ExternalInput")
    d_ln2b = nc.dram_tensor("ln2b", [L, E], F32, kind="ExternalInput")
    d_bo = nc.dram_tensor("bo", [L, E], F32, kind="ExternalInput")
    d_b1 = nc.dram_tensor("b1", [L, FF], F32, kind="ExternalInput")
    d_b2 = nc.dram_tensor("b2", [L, E], F32, kind="ExternalInput")
    d_lnfg = nc.dram_tensor("lnfg", [E], F32, kind="ExternalInput")
    d_lnfb = nc.dram_tensor("lnfb", [E], F32, kind="ExternalInput")
    d_out = nc.dram_tensor("logt", [NVT, P, S], BF16, kind="ExternalOutput")

    groups = [[0, 1, 2, 3], [4, 5, 6, 7]]

    with ExitStack() as ctx:
        tc = ctx.enter_context(tile.TileContext(nc, num_cores=NC))
        const = ctx.enter_context(tc.tile_pool(name="const", bufs=1))
        pp_x = ctx.enter_context(tc.tile_pool(name="xres", bufs=1))
        pp_pl = ctx.enter_context(tc.tile_pool(name="planes", bufs=1))
        tp = ctx.enter_context(tc.tile_pool(name="tp", bufs=1))
        wp = ctx.enter_context(tc.tile_pool(name="wstream", bufs=1))
        ap_p = ctx.enter_context(tc.tile_pool(name="attn", bufs=1))
        pacc = ctx.enter_context(tc.tile_pool(name="pacc", bufs=1, space="PSUM"))
        dram = ctx.enter_context(tc.tile_pool(name="ccdram", bufs=2, space="DRAM"))
        pools = {"pacc": pacc, "tp": tp}

        ident = const.tile([P, P], F32, name="ident")
        make_identity(nc, ident[:])
        ones_b = const.tile([P, P], BF16, name="ones_b")
        nc.vector.memset(ones_b[:], 1.0)
        ones8 = const.tile([P, 2, HD], FP8, name="ones8")
        nc.vector.memset(ones8[:], 1.0)
        zeros_b = const.tile([P, S], BF16, name="zeros_b")
        nc.vector.memset(zeros_b[:], 0.0)
        eps_t = const.tile([P, 1], F32, name="eps_t")
        nc.vector.memset(eps_t[:], EPS)

        def ldvec(dt_ap, n, name):
            t = const.tile([P, n], F32, tag=name, name=name)
            nc.sync.dma_start(out=t[:], in_=dt_ap.rearrange("(a p) -> p a", p=P))
            return t

        t_ln1g = [ldvec(d_ln1g.ap()[l], NE, f"ln1g{l}") for l in range(L)]
        t_ln1b = [ldvec(d_ln1b.ap()[l], NE, f"ln1b{l}") for l in range(L)]
        t_ln2g = [ldvec(d_ln2g.ap()[l], NE, f"ln2g{l}") for l in range(L)]
        t_ln2b = [ldvec(d_ln2b.ap()[l], NE, f"ln2b{l}") for l in range(L)]
        t_bo = [ldvec(d_bo.ap()[l], NE, f"bo{l}") for l in range(L)]
        t_b1 = [ldvec(d_b1.ap()[l], NFT, f"b1{l}") for l in range(L)]
        t_b2 = [ldvec(d_b2.ap()[l], NE, f"b2{l}") for l in range(L)]
        t_lnfg = ldvec(d_lnfg.ap(), NE, "lnfg")
        t_lnfb = ldvec(d_lnfb.ap(), NE, "lnfb")

        # diagonal-quad masks (per-core data): [slot, 128k, 4kb, 128q]
        mask_t = []
        for i in range(4):
            m = const.tile([P, 8, P], FP8, tag=f"mskd{i}", name=f"mskd{i}")
            nc.sync.dma_start(out=m[:, 0:4], in_=d_msk.ap()[i])
            nc.sync.dma_start(out=m[:, 4:8], in_=d_msk.ap()[i])
            mask_t.append(m)

        # residual stream: 8 bf16 [128, S] tiles
        xT = [pp_x.tile([P, S], BF16, tag=f"x{e}", name=f"x{e}") for e in range(NE)]

        # ---- embedding gather + transpose ----
        idx_t = const.tile([P, S // P], I32, name="idx_t")
        nc.sync.dma_start(out=idx_t[:], in_=d_idx.ap().rearrange("(g p) -> p g", p=P))
        for g in range(S // P):
            xg = tp.tile([P, E], F32, tag="embg", name="embg", bufs=1)
            nc.gpsimd.indirect_dma_start(
                out=xg[:], out_offset=None, in_=d_emb.ap(),
                in_offset=bass.IndirectOffsetOnAxis(ap=idx_t[:, g:g + 1], axis=0))
            for e in range(NE):
                pst = pacc.tile([P, S], F32, tag="ps_a", name="tpose", bufs=2)
                nc.tensor.transpose(pst[:, 0:P], xg[:, e * P:(e + 1) * P], ident[:])
                nc.vector.tensor_copy(xT[e][:, g * P:(g + 1) * P], pst[:, 0:P])

        # x fp8 planes for matmul rhs: 4 tiles [128, 2, S]
        # x planes carry 8*x so layer-0 embeddings (~0.02) clear the fp8
        # subnormal range; the 1/8 rides the existing k/q/v descale copies
        xP = [pp_pl.tile([P, 2, S], FP8, tag=f"xp{r}", name=f"xp{r}")
              for r in range(NE // 2)]
        for e in range(NE):
            nc.vector.tensor_scalar(xP[e // 2][:, e % 2, :], xT[e][:], 8.0,
                                    None, OP.mult)

        def proj_dr(d_w, l, rhs_planes, nm):
            """W-stationary fp8 DR projection -> list of psum [128,S] per m."""
            w = wp.tile([P, NE, NE // 2, 2, P], FP8, tag="wproj", name=f"w{nm}",
                        bufs=2)
            nc.sync.dma_start(out=w[:], in_=d_w.ap()[l])
            outs = []
            for m in range(NE):
                ps = pacc.tile([P, S], F32, tag="ps_a", name=f"ps{nm}", bufs=2)
                for kp in range(NE // 2):
                    nc.tensor.matmul(ps[:], lhsT=w[:, m, kp], rhs=rhs_planes[kp][:],
                                     start=(kp == 0), stop=(kp == NE // 2 - 1),
                                     perf_mode=DRM)
                outs.append(ps)
            return outs

        # ---- transformer layers ----
        for l in range(L):
            with tc.tile_pool(name="kv", bufs=1) as kvp, \
                 tc.tile_pool(name="qq", bufs=1) as qp, \
                 tc.tile_pool(name="oo", bufs=1) as op_:
                # K projection -> fp8 (x 1/32) -> per-half kv blobs
                # (half A = local token cols 0:256 = abs key quads 0,1)
                KB2 = E * (S // 2)
                CH2 = 2 * KB2
                kvin = [dram.tile([CH2], FP8, tag=f"kvin{a}", name=f"kvin{a}")
                        for a in range(2)]
                kreg = [kvin[a][:KB2].rearrange("(r s) -> r s", r=E, s=S // 2)
                        for a in range(2)]
                vreg = [kvin[a][KB2:].rearrange("(t e) -> t e", t=S // 2, e=E)
                        for a in range(2)]
                # k blob rows in consumer order (row = 8p + 2hh + i, with the
                # Wk output columns host-permuted to match) so each m-tile
                # writes one contiguous row block per half
                kps = proj_dr(d_wk, l, xP, "k")
                for m in range(NE):
                    kl = tp.tile([P, S], FP8, tag="kl", name="kl", bufs=2)
                    nc.vector.tensor_scalar(kl[:], kps[m][:], 1.0 / (WS * 8), None, OP.mult)
                    for a in range(2):
                        nc.sync.dma_start(
                            out=kreg[a][m * P:(m + 1) * P, :],
                            in_=kl[:, a * (S // 2):(a + 1) * (S // 2)])
                # V projection half A (token blocks 0,1) -> AllGather A
                wv = wp.tile([P, NE, E], BF16, tag="wv", name="wv", bufs=1)
                nc.sync.dma_start(out=wv[:], in_=d_wv.ap()[l])
                kvout = [dram.tile([CH, CH2], FP8, tag=f"kvout{a}",
                                   name=f"kvout{a}") for a in range(2)]

                def vproj_half(a):
                    for tb in (2 * a, 2 * a + 1):
                        for hf in range(2):
                            ps = pacc.tile([P, S], F32, tag="ps_a", name="psv",
                                           bufs=2)
                            for k in range(NE):
                                nc.tensor.matmul(
                                    ps[:], lhsT=xT[k][:, tb * P:(tb + 1) * P],
                                    rhs=wv[:, k, hf * S:(hf + 1) * S],
                                    start=(k == 0), stop=(k == NE - 1))
                            vl = tp.tile([P, S], FP8, tag="vl", name="vl", bufs=2)
                            nc.vector.tensor_scalar(vl[:], ps[:], 8.0,
                                                    None, OP.mult)
                            nc.sync.dma_start(
                                out=vreg[a][(tb % 2) * P:(tb % 2 + 1) * P,
                                            hf * S:(hf + 1) * S],
                                in_=vl[:])

                def gather_half(a):
                    nc.gpsimd.collective_compute(
                        "AllGather", OP.bypass, replica_groups=groups,
                        ins=[kvin[a][:]], outs=[kvout[a][:]])

                ktile = ap_p.tile([P, 4, 2, 4, 4, P], FP8, tag="kt", name="kt",
                                  bufs=1)
                vt = [ap_p.tile([P, 2, E], FP8, tag=f"vt{r}", name=f"vt{r}",
                                bufs=1) for r in range(NE)]

                def load_half(a):
                    # k/v slab loads on the Act DMA queue so they don't queue
                    # behind SP-issued q/blob writes
                    kr = kvout[a][:, :KB2].rearrange(
                        "c (p hh i us) -> c p hh i us", p=P, hh=4, i=2,
                        us=S // 2)
                    for c in range(4):
                        nc.gpsimd.dma_start(
                            out=ktile[:, :, :, c, 2 * a:2 * a + 2],
                            in_=kr[c].rearrange("p hh i (u s) -> p hh i u s",
                                                u=2, s=P))
                    for r in range(4 * a, 4 * a + 4):
                        for pl in range(2):
                            kb = 2 * r + pl
                            u, c = kb // 4, kb % 4
                            u2 = u % 2
                            nc.gpsimd.dma_start(
                                out=vt[r][:, pl],
                                in_=kvout[a][c,
                                             KB2 + u2 * P * E:KB2 + (u2 + 1) * P * E]
                                .rearrange("(p s) -> p s", p=P))

                vproj_half(0)
                gather_half(0)

                # Q projection (PE busy during AllGather A) -> DRAM roundtrip
                # for the [32, 2, q] plane layout
                qbuf = dram.tile([E, S], FP8, tag="qbuf", name="qbuf")
                qps = proj_dr(d_wq, l, xP, "q")
                for m in range(NE):
                    ql = tp.tile([P, S], FP8, tag="ql", name="ql", bufs=2)
                    nc.vector.tensor_scalar(ql[:], qps[m][:], 1.0 / (WS * 8),
                                            None, OP.mult)
                    nc.sync.dma_start(out=qbuf[m * P:(m + 1) * P, :], in_=ql[:])
                # q tile [128 (4g x 32d), 4hh, 2i, 512]; head h = 4*hh + g;
                # blob row = 8p + 2hh + i -> single balanced DMA
                qtile = qp.tile([P, 4, 2, S], FP8, tag="qt", name="qt")
                nc.sync.dma_start(
                    out=qtile[:],
                    in_=qbuf[:].rearrange("(p hh i) s -> p hh i s",
                                          p=P, hh=4, i=2))

                vproj_half(1)
                load_half(0)
                gather_half(1)
                load_half(1)

                # attention: slot-outer so quad-0/1 work overlaps AllGather B.
                # Head pairs (h, h+8) share a partition group -> one batched
                # exp per quad pair (halves the per-instruction PSUM-access
                # penalty on the Act engine).
                oT = [op_.tile([P, 4, P], BF16, tag=f"ot{m}", name=f"ot{m}")
                      for m in range(NE)]
                og_t = [op_.tile([64, 4, P], BF16, tag=f"og{oh}", name=f"og{oh}")
                        for oh in range(NHP)]
                for i in range(4):
                    nkbp = 2 * (i + 1)
                    for hp2 in range(NHP):
                        g, hhl = hp2 % 4, hp2 // 4
                        pd = [pacc.tile([64, 2, P], F32, tag="ps_v",
                                        name="pavden", bufs=2) for _ in range(2)]
                        for u in range(i + 1):
                            sc = pacc.tile([P, 8, P], F32, tag="ps_s",
                                           name="sc", bufs=2)
                            for half in range(2):
                                hh = hhl + 2 * half
                                for c in range(4):
                                    nc.tensor.matmul(
                                        sc[:, 4 * half + c],
                                        lhsT=ktile[32 * g:32 * (g + 1), hh, :, c, u],
                                        rhs=qtile[32 * g:32 * (g + 1), hh, :,
                                                  i * P:(i + 1) * P],
                                        start=True, stop=True, perf_mode=DRM,
                                        tile_position=(32 * g, 0))
                            pa = ap_p.tile([P, 8, P], FP8, tag="pa",
                                           name="pa", bufs=4)
                            nc.scalar.activation(pa[:], sc[:], AF.Exp,
                                                 scale=HD ** -0.5)
                            if u == i:
                                nc.vector.tensor_tensor(
                                    out=pa[:], in0=pa[:], in1=mask_t[i][:],
                                    op=OP.mult)
                            for half in range(2):
                                h = hp2 + 8 * half
                                for r2 in range(2):
                                    kbp = 2 * u + r2
                                    first = (kbp == 0)
                                    last = (kbp == nkbp - 1)
                                    nc.tensor.matmul(
                                        pd[half][:, 0],
                                        lhsT=vt[kbp][:, :, h * HD:(h + 1) * HD],
                                        rhs=pa[:, 4 * half + 2 * r2:
                                               4 * half + 2 * r2 + 2],
                                        start=first, stop=last, perf_mode=DRM,
                                        skip_group_check=True)
                                    nc.tensor.matmul(
                                        pd[half][:, 1],
                                        lhsT=ones8[:, :, :],
                                        rhs=pa[:, 4 * half + 2 * r2:
                                               4 * half + 2 * r2 + 2],
                                        start=first, stop=last, perf_mode=DRM,
                                        skip_group_check=True)
                        for half in range(2):
                            h = hp2 + 8 * half
                            rec = tp.tile([64, P], F32, tag="rec", name="rec",
                                          bufs=2)
                            nc.vector.reciprocal(rec[:], pd[half][:, 1])
                            # o = pav*rec -> bf16; odd heads stage + DMA-shift
                            # (DR matmuls must write psum partition base 0)
                            if h % 2 == 0:
                                nc.vector.tensor_tensor(
                                    out=oT[h // 2][0:64, i],
                                    in0=pd[half][:, 0], in1=rec[:], op=OP.mult)
                            else:
                                nc.vector.tensor_tensor(
                                    out=og_t[h // 2][:, i],
                                    in0=pd[half][:, 0], in1=rec[:], op=OP.mult)
                for h in range(1, H, 2):
                    nc.sync.dma_start(out=oT[h // 2][64:128], in_=og_t[h // 2][:])

                # output projection (bf16 for accuracy) + bias + residual
                sum_t = []
                wo = None
                for m in range(NE):
                    if m % 4 == 0:
                        wo = wp.tile([P, 4, NE, P], BF16, tag="w1",
                                     name="wo", bufs=2)
                        nc.sync.dma_start(out=wo[:], in_=d_wo.ap()[l, :, m:m + 4])
                    ps = pacc.tile([P, S], F32, tag="ps_a", name="pso", bufs=2)
                    for k in range(NE):
                        nc.tensor.matmul(ps[:], lhsT=wo[:, m % 4, k],
                                         rhs=oT[k][:], start=(k == 0),
                                         stop=(k == NE - 1))
                    xb = tp.tile([P, S], BF16, tag="xb", name="xb", bufs=3)
                    nc.vector.tensor_scalar(xb[:], xT[m][:], t_bo[l][:, m:m + 1],
                                            None, OP.add)
                    st = op_.tile([P, S], BF16, tag=f"st{m}", name=f"st{m}")
                    nc.vector.scalar_tensor_tensor(
                        out=st[:], in0=ps[:], scalar=0.125, in1=xb[:],
                        op0=OP.mult, op1=OP.add)
                    sum_t.append(st)

                def ln1_out(e, t2, g, b):
                    nc.scalar.activation(xT[e][:], t2[:], AF.Identity, bias=b, scale=g)
                _ln(nc, pools, sum_t, ln1_out, t_ln1g[l], t_ln1b[l], ones_b, eps_t)

            # FFN (bf16 for accuracy: fp8 here injects ~5%/layer into the
            # residual, which blows the 2e-2 budget)
            with tc.tile_pool(name="ht", bufs=1) as hp_:
                hT = [hp_.tile([P, S], BF16, tag=f"h{f2}", name=f"h{f2}")
                      for f2 in range(NFT)]
                w1 = None
                for f in range(NFT):
                    if f % 4 == 0:
                        w1 = wp.tile([P, 4, NE, P], BF16, tag="w1",
                                     name="w1", bufs=2)
                        nc.sync.dma_start(out=w1[:],
                                          in_=d_w1.ap()[l, :, f:f + 4])
                    ps = pacc.tile([P, S], F32, tag="ps_a", name="psf", bufs=2)
                    for k in range(NE):
                        nc.tensor.matmul(ps[:], lhsT=w1[:, f % 4, k],
                                         rhs=xT[k][:], start=(k == 0),
                                         stop=(k == NE - 1))
                    nc.vector.scalar_tensor_tensor(
                        out=hT[f][:], in0=ps[:],
                        scalar=t_b1[l][:, f:f + 1], in1=zeros_b[:],
                        op0=OP.add, op1=OP.max)
                w2 = None
                sum2 = []
                for m in range(NE):
                    if True:
                        w2 = wp.tile([P, 1, NFT, P], BF16, tag="w2",
                                     name="w2", bufs=2)
                        nc.sync.dma_start(out=w2[:],
                                          in_=d_w2.ap()[l, :, m:m + 1])
                    ps = pacc.tile([P, S], F32, tag="ps_a", name="ps2", bufs=2)
                    for k in range(NFT):
                        nc.tensor.matmul(ps[:], lhsT=w2[:, 0, k],
                                         rhs=hT[k][:], start=(k == 0),
                                         stop=(k == NFT - 1))
                    xb = tp.tile([P, S], BF16, tag="xb2", name="xb2", bufs=3)
                    nc.vector.tensor_scalar(xb[:], xT[m][:], t_b2[l][:, m:m + 1],
                                            None, OP.add)
                    st = tp.tile([P, S], BF16, tag=f"s2{m}", name=f"s2{m}")
                    nc.vector.scalar_tensor_tensor(
                        out=st[:], in0=ps[:], scalar=1.0, in1=xb[:],
                        op0=OP.mult, op1=OP.add)
                    sum2.append(st)

                if l < L - 1:
                    def ln2_out(e, t2, g, b):
                        nc.scalar.activation(xT[e][:], t2[:], AF.Identity, bias=b,
                                             scale=g)
                        nc.vector.tensor_scalar(xP[e // 2][:, e % 2, :],
                                                xT[e][:], 8.0, None, OP.mult)
                    _ln(nc, pools, sum2, ln2_out, t_ln2g[l], t_ln2b[l], ones_b,
                        eps_t)
                else:
                    # fuse LN2(last) and LNf: LN(LN(x)) with composed affine is
                    # NOT equal in general; do them separately.
                    def ln2_out(e, t2, g, b):
                        nc.scalar.activation(xT[e][:], t2[:], AF.Identity, bias=b,
                                             scale=g)
                    _ln(nc, pools, sum2, ln2_out, t_ln2g[l], t_ln2b[l], ones_b,
                        eps_t)

        # ---- final LN + lm_head (bf16) ----
        with tc.tile_pool(name="lmx", bufs=1) as lmp:
            xB = [lmp.tile([P, S], BF16, tag=f"xlm{e}", name=f"xlm{e}")
                  for e in range(NE)]

            def lnf_out(e, t2, g, b):
                nc.scalar.activation(xB[e][:], t2[:], AF.Identity, bias=b, scale=g)
            _ln(nc, pools, xT, lnf_out, t_lnfg, t_lnfb, ones_b, eps_t)

            for vg in range(NVT // 2):
                w = wp.tile([P, 2, NE, P], BF16, tag="wlm", name="wlm", bufs=2)
                nc.sync.dma_start(out=w[:], in_=d_wlm.ap()[vg])
                lg = tp.tile([P, 2, S], BF16, tag="lg", name="lg", bufs=2)
                for v2 in range(2):
                    ps = pacc.tile([P, S], F32, tag="ps_a", name="pslm", bufs=2)
                    for k in range(NE):
                        nc.tensor.matmul(ps[:], lhsT=w[:, v2, k], rhs=xB[k][:],
                                         start=(k == 0), stop=(k == NE - 1))
                    if v2 == 0:
                        nc.vector.tensor_copy(lg[:, v2], ps[:])
                    else:
                        nc.scalar.copy(lg[:, v2], ps[:])
                nc.sync.dma_start(out=d_out.ap()[2 * vg:2 * vg + 2]
                                  .rearrange("v p s -> p v s"), in_=lg[:])

    nc.compile()
    return nc


_CACHED = {}


def _pack_dr_w(w, m_tiles):
    """[E_in, E_out] -> [P, m, kp, 2, P] fp8 with x32 prescale."""
    ein, eout = w.shape
    kp = ein // 256
    arr = (w * WS).reshape(kp, 2, P, m_tiles, P)
    # w[kpair*256 + i*128 + p, m*128 + c] -> [p, m, kp, i, c]
    arr = arr.transpose(2, 3, 0, 1, 4)
    return np.ascontiguousarray(arr).astype(E4NP)


def _prep_weights(inputs):
    f32 = np.float32
    Wq, Wk, Wv = (np.asarray(inputs[k], dtype=f32) for k in ("Wq", "Wk", "Wv"))
    # [L,H,E,HD] -> [L, E, H*HD]
    wq = Wq.transpose(0, 2, 1, 3).reshape(L, E, E)
    wk = Wk.transpose(0, 2, 1, 3).reshape(L, E, E)
    wv = Wv.transpose(0, 2, 1, 3).reshape(L, E, E)
    Wo = np.asarray(inputs["Wo"], dtype=f32)
    W1 = np.asarray(inputs["W1"], dtype=f32)
    W2 = np.asarray(inputs["W2"], dtype=f32)
    Wlm = np.asarray(inputs["Wlm"], dtype=f32)

    # permute k/q output features into blob-row order r = 8p + 2hh + i
    r = np.arange(E)
    p_, q8 = r // 8, r % 8
    hh_, i_ = q8 // 2, q8 % 2
    g_, d_ = p_ // 32, p_ % 32
    perm = (4 * hh_ + g_) * 64 + i_ * 32 + d_
    wq8 = np.stack([_pack_dr_w(wq[l][:, perm], NE) for l in range(L)])
    wk8 = np.stack([_pack_dr_w(wk[l][:, perm], NE) for l in range(L)])
    bf = ml_dtypes.bfloat16
    # Wv as moving rhs: [p, k, col]
    wvb = np.stack([np.ascontiguousarray(
        wv[l].reshape(NE, P, E).transpose(1, 0, 2)).astype(bf)
        for l in range(L)])
    wob = np.stack([np.ascontiguousarray(
        Wo[l].reshape(NE, P, NE, P).transpose(1, 2, 0, 3)).astype(bf)
        for l in range(L)])
    # W1 [E, FF] -> [P, f, k, P]; W2 [FF, E] -> [P, m, k, P]
    w1b = np.stack([np.ascontiguousarray(
        W1[l].reshape(NE, P, NFT, P).transpose(1, 2, 0, 3)).astype(bf)
        for l in range(L)])
    w2b = np.stack([np.ascontiguousarray(
        W2[l].reshape(NFT, P, NE, P).transpose(1, 2, 0, 3)).astype(bf)
        for l in range(L)])
    # Wlm [E, V] -> [vg, P, 2, k, P]
    wlm2 = np.ascontiguousarray(
        Wlm.reshape(NE, P, NVT // 2, 2, P).transpose(2, 1, 3, 0, 4)).astype(bf)

    return {
        "wq8": wq8, "wk8": wk8, "wob": wob, "wvb": wvb, "w1b": w1b, "w2b": w2b,
        "wlm2": wlm2,
        "emb": np.ascontiguousarray(inputs["emb"]).astype(f32),
        "ln1g": np.ascontiguousarray(inputs["ln1_g"]).astype(f32),
        "ln1b": np.ascontiguousarray(inputs["ln1_b"]).astype(f32),
        "ln2g": np.ascontiguousarray(inputs["ln2_g"]).astype(f32),
        "ln2b": np.ascontiguousarray(inputs["ln2_b"]).astype(f32),
        "bo": np.ascontiguousarray(inputs["bo"]).astype(f32),
        "b1": np.ascontiguousarray(inputs["b1"]).astype(f32),
        "b2": np.ascontiguousarray(inputs["b2"]).astype(f32),
        "lnfg": np.ascontiguousarray(inputs["lnf_g"]).astype(f32),
        "lnfb": np.ascontiguousarray(inputs["lnf_b"]).astype(f32),
    }


def kernel(**inputs):
    if "nc" not in _CACHED:
        _CACHED["nc"] = build_program()
    nc = _CACHED["nc"]

    shared = _prep_weights(inputs)
    index = np.asarray(inputs["index"])

    kpos = np.arange(P)
    in_maps = []
    perms = []
    for c in range(NC):
        b, j = c // CH, c % CH
        perm = np.concatenate([np.arange((4 * i + j) * P, (4 * i + j + 1) * P)
                               for i in range(4)])
        perms.append(perm)
        # diag-quad masks [slot, k, kb_in_quad (c2), q]
        m = np.zeros((4, P, 4, P), np.float32)
        for i in range(4):
            for c2 in range(4):
                if c2 < j:
                    m[i, :, c2, :] = 1.0
                elif c2 == j:
                    m[i, :, c2, :] = (kpos[:, None] <= kpos[None, :])
        im = dict(shared)
        im["maskd"] = m.astype(E4NP)
        im["idx"] = np.ascontiguousarray(index[b, perm]).astype(np.int32)
        in_maps.append(im)

    res = bass_utils.run_bass_kernel_spmd(nc, in_maps, core_ids=list(range(NC)))
    blm = np.asarray(inputs["blm"], dtype=np.float32)
    out = np.zeros((B, T, V), np.float32)
    for c in range(NC):
        b = c // CH
        lg = res.results[c]["logt"].reshape(V, S).astype(np.float32)
        out[b, perms[c], :] = lg.T
    out += blm[None, None, :]
    return out
